# revision 17
# baseline (speedup 1.0000x reference)
"""Single-head causal attention (B=8, T=2048, D=1024, fp32 I/O) on 8 trn2
NeuronCores, data-parallel over batch (one batch element per core).

Per-core algorithm (QKV/AV matmuls bf16, scores fp8 DoubleRow, fp32 PSUM):
  xT   = transpose(cast_bf16(x))   t0-7: PE-transpose; t8-15: DRAM+xbar
  qT   = Wq^T-stationary matmuls  -> (e, t) layout, stored fp8e4
  kT   = same                                     -> (e, t) layout, fp8e4
  v    = xT-stationary matmuls    -> (t, e) layout, bf16
  S^T  block (j, i) = kT(:,j)-stationary @ qT, fp8 DoubleRow (2 e-blocks
         per matmul, 2x PE throughput; rel-err ~1.2e-2 vs 2e-2 budget)
  E^T  = exp(S^T / 32)   (no max-subtraction needed: |S/32| <~ 2)
         diagonal blocks masked by an upper-triangular 0/1 multiply
  rowsum_i = ones-matmul with E^T stationary      (PSUM accumulation over j)
  out  = (E^T-stationary @ v) * (1/rowsum)        per-partition scalar scale

The softmax normalization is applied to the AV output instead of to the
weights, so no transposes of the (T, T) attention matrix are ever needed.
Score spans are aligned to the causal diagonal so no masked block is ever
computed except the triangular diagonal blocks themselves.

Schedule: x tiles 0-1 are dispatched on the sync ring ahead of all weight
traffic; masks build first on gpsimd. The xbar wait for t-spans 2-3 is
covered by the v-production for t-blocks 0-7 plus the (now cheap) fp8
early scores for i-spans 0-1. Attention spans run in order 1,2,3,0 with
span 0's AV blocks descending, so the tail-gating chain is a single
j-block; its output scales+DMAs are split in half and dispatched across
engines to minimize the exposed tail.
"""
import sys
import types

import numpy as np

import concourse.bass as bass
import concourse.mybir as mybir
import concourse.tile as tile
from concourse.bass_utils import run_bass_kernel_spmd
from concourse.masks import make_identity, make_upper_triangular

B, T, D = 8, 2048, 1024
P = 128
TB = T // P        # 16 t-blocks
DBLK = D // P      # 8 d/e-blocks
NTS = T // 512     # 4 t-spans of 512
NES = D // 512     # 2 e-spans of 512
SCALE = 1.0 / 32.0  # 1/sqrt(D)

F32 = mybir.dt.float32
BF16 = mybir.dt.bfloat16
F8 = mybir.dt.float8e4
DR = mybir.MatmulPerfMode.DoubleRow


def _install_ntff_hook():
    """Optional: register the axon NTFF profiling hook (the agent image's
    antenv lacks axon_hooks). Lets BASS_TRACE=1 produce exec_time_ns."""
    try:
        import antenv

        if "antenv.axon_hooks" in sys.modules:
            return
        mod = types.ModuleType("antenv.axon_hooks")
        _hook = [None]
        mod.set_axon_ntff_profile_hook = lambda h: _hook.__setitem__(0, h)
        mod.get_axon_ntff_profile_hook = lambda: _hook[0]
        sys.modules["antenv.axon_hooks"] = mod
        antenv.axon_hooks = mod
        from trn_agent_boot.trn_boot import _ntff_profile_via_ctypes

        mod.set_axon_ntff_profile_hook(
            _ntff_profile_via_ctypes("/opt/axon/libaxon_pjrt.so")
        )
    except Exception:
        pass


_install_ntff_hook()


def _split_multi_waits(nc: bass.Bass):
    """Walrus on this stack fits only ONE sync-wait per instruction, but
    Tile emits several on multi-producer instructions. Hoist the extra waits
    onto single-wait NoOps placed just before, on the same engine — the
    per-engine streams are in-order, so semantics are identical."""
    n_split = 0
    for fn in nc.m.functions:
        for bb in fn.blocks:
            out = []
            changed = False
            for inst in bb.instructions:
                si = inst.sync_info
                waits = list(si.on_wait) if si is not None and si.on_wait else []
                if len(waits) > 1:
                    for w in waits[:-1]:
                        nop = mybir.InstNoOp(
                            name=nc.get_next_instruction_name(),
                            engine=inst.engine,
                            ins=[],
                            outs=[],
                            sync_info=mybir.SyncInfo(on_wait=[w], on_update=[]),
                            bass_nofuse=True,
                        )
                        out.append(nop)
                    inst.sync_info = mybir.SyncInfo(
                        on_wait=[waits[-1]],
                        on_update=list(si.on_update or []),
                    )
                    changed = True
                    n_split += 1
                out.append(inst)
            if changed:
                bb.instructions = out
    return n_split


def _emit(nc: bass.Bass):
    x = nc.dram_tensor("x", [T, D], F32, kind="ExternalInput").ap()
    Wq = nc.dram_tensor("Wq", [D, D], F32, kind="ExternalInput").ap()
    Wk = nc.dram_tensor("Wk", [D, D], F32, kind="ExternalInput").ap()
    Wv = nc.dram_tensor("Wv", [D, D], F32, kind="ExternalInput").ap()
    out = nc.dram_tensor("out", [T, D], F32, kind="ExternalOutput").ap()

    with tile.TileContext(nc) as tc:
        from contextlib import ExitStack

        with ExitStack() as ctx:
            persist = ctx.enter_context(tc.tile_pool(name="persist", bufs=1))
            psum = ctx.enter_context(tc.tile_pool(name="psum", bufs=6, space="PSUM"))

            # ---- persistent SBUF tensors (survive the whole kernel) ----
            qT = persist.tile([P, DBLK, T], F8)         # (e, t)
            kT = persist.tile([P, DBLK, T], F8)         # (e, t)
            vsb = persist.tile([P, TB, D], BF16)        # (t, e)
            # E^T tiles for i-spans 0-1 (computed early, inside phase B,
            # to fill the PE bubble while ts2/ts3 xT transposes land)
            etE = persist.tile([P, 12, 512], BF16)      # (j, i) blocks
            ones = persist.tile([P, 1], BF16)
            triu = persist.tile([P, P], BF16)
            ident = persist.tile([P, P], BF16)

            # ============ Phase A+B: load/cast/transpose + QKV ==========
            # All 16 x tiles are PE-transposed (no DRAM bf16 store + xbar
            # reload): the 8 cores run the same schedule concurrently and
            # share HBM, so per-core DMA saturates at ~200GB/s in phase B —
            # dropping the 8.4MB store+reload roundtrip is worth far more
            # than the ~3.4us of PE transpose work it adds.
            with tc.tile_pool(name="qkvp", bufs=1) as qkvp, \
                 tc.tile_pool(name="staging", bufs=4) as staging:
                xT = qkvp.tile([P, DBLK, T], BF16)          # (d, t)
                # Wq/Wk share two 16KB slots; Wv reuses Wq's slot after the
                # last q matmul (q-ts3) has read it.
                wq_bf = qkvp.tile([P, DBLK, D], BF16, tag="wbf", bufs=2)
                wk_bf = qkvp.tile([P, DBLK, D], BF16, tag="wbf", bufs=2)
                wv_bf = qkvp.tile([P, DBLK, D], BF16, tag="wbf", bufs=2)

                # x tiles 0-3 dispatched on the sync ring FIRST (its NEFF
                # preamble retires earliest) and tiles 4-7 on the gpsimd
                # ring (slot-throttled by the xs32 tag so their queue entries
                # trail x0-3 and the first Wq chunks), all AHEAD of the bulk
                # weight traffic, so the transpose chain is never queued
                # behind Wq on the DMA queues.
                pre_stage = {}
                for tb in range(4):
                    sf = staging.tile([P, D], F32, tag="xs32", bufs=4)
                    if tb == 0:
                        # halves: the first cast starts ~0.6us earlier
                        nc.sync.dma_start(
                            out=sf[:, 0:512], in_=x[tb * P:(tb + 1) * P, 0:512]
                        )
                        nc.sync.dma_start(
                            out=sf[:, 512:D], in_=x[tb * P:(tb + 1) * P, 512:D]
                        )
                    else:
                        nc.sync.dma_start(out=sf, in_=x[tb * P:(tb + 1) * P, :])
                    pre_stage[tb] = sf

                # masks build first on the gpsimd ring (identity gates the
                # very first PE transpose; ready well before the first x
                # cast lands).
                make_identity(nc, ident)
                nc.vector.memset(ones, 1.0)
                make_upper_triangular(nc, triu, val=1.0, diag=True)

                for tb in range(4, 8):
                    sf = staging.tile([P, D], F32, tag="xs32", bufs=4)
                    nc.gpsimd.dma_start(out=sf, in_=x[tb * P:(tb + 1) * P, :])
                    pre_stage[tb] = sf

                def x_chain_pe(tb):
                    """Cast pre-staged x row-block tb to bf16, PE-transpose
                    the 8 [128,128] sub-blocks into xT (fast path for the
                    first two t-spans, before the xbar pipeline warms up)."""
                    stage_f32 = pre_stage[tb]
                    stage_bf = staging.tile([P, D], BF16, tag="xsbf", bufs=5)
                    nc.vector.tensor_copy(stage_bf[:, 0:512], stage_f32[:, 0:512])
                    nc.scalar.copy(stage_bf[:, 512:D], stage_f32[:, 512:D])
                    for db in range(DBLK):
                        # transpose as a REGULAR matmul (x-block stationary,
                        # identity moving): out = x_blk.T @ I. Issues ~2.5x
                        # faster than transpose_mode and warms HAM.
                        pst = psum.tile([P, P], F32, tag="small", bufs=2)
                        nc.tensor.matmul(
                            pst,
                            lhsT=stage_bf[:, db * P:(db + 1) * P],
                            rhs=ident,
                            start=True,
                            stop=True,
                        )
                        nc.any.tensor_copy(xT[:, db, tb * P:(tb + 1) * P], pst)

                xscast = [0]

                def x_late_dma(tb):
                    """Dispatch the load for a late x row-block (8-15);
                    cast + quad-transpose happen later, data-paced."""
                    stage_f32 = staging.tile([P, D], F32, tag="xs32b", bufs=4)
                    nc.sync.dma_start(
                        out=stage_f32, in_=x[tb * P:(tb + 1) * P, :]
                    )
                    return stage_f32

                def x_late_cast(stage_f32):
                    stage_bf = staging.tile([P, D], BF16, tag="xsbf", bufs=5)
                    if xscast[0] % 2 == 0:
                        nc.vector.tensor_copy(stage_bf, stage_f32)
                    else:
                        nc.scalar.copy(stage_bf, stage_f32)
                    xscast[0] += 1
                    return stage_bf

                def x_quad_transpose(stage_bfs, tb0):
                    """Transpose 4 cast tiles (t-blocks tb0..tb0+3) with one
                    [128,512] PSUM bank + one 512-wide evacuation per db —
                    4x fewer evacuations than per-block small-psum
                    transposes."""
                    for db in range(DBLK):
                        ps = psum.tile([P, 512], F32, tag="big")
                        for i, sbf in enumerate(stage_bfs):
                            nc.tensor.matmul(
                                ps[:, i * P:(i + 1) * P],
                                lhsT=sbf[:, db * P:(db + 1) * P],
                                rhs=ident,
                                start=True,
                                stop=True,
                            )
                        nc.any.tensor_copy(
                            xT[:, db, tb0 * P:(tb0 + 4) * P], ps
                        )

                wcast = [0]

                def w_dma(w_dram, db, dma_engine=None):
                    """Dispatch one weight chunk load; the cast is emitted
                    separately (w_cast) at a point in the DVE/ACT streams
                    where the data has surely landed, so a data-waiting cast
                    never head-of-line-blocks PSUM evacuations."""
                    stage_f32 = staging.tile([P, D], F32, tag="ws32", bufs=4)
                    (dma_engine or nc.sync).dma_start(
                        out=stage_f32, in_=w_dram[db * P:(db + 1) * P, :]
                    )
                    return stage_f32

                def w_cast(stage_f32, w_sb, db):
                    """Casts alternate DVE/ACT so neither paces the stream."""
                    if wcast[0] % 2 == 0:
                        nc.vector.tensor_copy(w_sb[:, db, :], stage_f32)
                    else:
                        nc.scalar.copy(w_sb[:, db, :], stage_f32)
                    wcast[0] += 1

                def w_chain(w_dram, w_sb, db, dma_engine=None):
                    w_cast(w_dma(w_dram, db, dma_engine), w_sb, db)

                def score_exp(jb, i0, L, et_ap):
                    """S^T block row jb over i in [i0, i0+L): fp8 DoubleRow
                    matmuls (2 e-blocks per instruction), exp (scaled),
                    diagonal mask if the span starts on the causal
                    diagonal."""
                    ps = psum.tile([P, 512], F32, tag="big")
                    for m in range(DBLK // 2):
                        nc.tensor.matmul(
                            ps[:, 0:L],
                            lhsT=kT[:, 2 * m:2 * m + 2, jb * P:(jb + 1) * P],
                            rhs=qT[:, 2 * m:2 * m + 2, i0:i0 + L],
                            start=(m == 0),
                            stop=(m == DBLK // 2 - 1),
                            perf_mode=DR,
                        )
                    nc.scalar.activation(
                        et_ap, ps[:, 0:L],
                        mybir.ActivationFunctionType.Exp, scale=SCALE,
                    )
                    if jb * P >= i0:  # diagonal block leads this span
                        nc.vector.tensor_mul(
                            et_ap[:, 0:P], et_ap[:, 0:P], triu
                        )

                def qk_group(w_sb, dstT, ts):
                    for eb in range(DBLK):
                        ps = psum.tile([P, 512], F32, tag="big")
                        for db in range(DBLK):
                            nc.tensor.matmul(
                                ps,
                                lhsT=w_sb[:, db, eb * P:(eb + 1) * P],
                                rhs=xT[:, db, ts * 512:(ts + 1) * 512],
                                start=(db == 0),
                                stop=(db == DBLK - 1),
                            )
                        nc.any.tensor_copy(
                            dstT[:, eb, ts * 512:(ts + 1) * 512], ps
                        )

                def v_group(tb):
                    # v: out[t(128), e(512)] = sum_d xT[d, t]-stat @ W[d, e]
                    for es in range(NES):
                        ps = psum.tile([P, 512], F32, tag="big")
                        for db in range(DBLK):
                            nc.tensor.matmul(
                                ps,
                                lhsT=xT[:, db, tb * P:(tb + 1) * P],
                                rhs=wv_bf[:, db, es * 512:(es + 1) * 512],
                                start=(db == 0),
                                stop=(db == DBLK - 1),
                            )
                        nc.any.tensor_copy(vsb[:, tb, es * 512:(es + 1) * 512], ps)

                # Wq DMAs dispatch right after the x0-3 pre-dispatches on
                # sync; their casts are interleaved BETWEEN the transpose
                # chains so a Wq cast waiting on data never blocks an x cast
                # or a transpose evacuation queued behind it on DVE/ACT.
                wq_stage = [w_dma(Wq, db) for db in range(DBLK)]
                x_chain_pe(0)
                x_chain_pe(1)
                w_cast(wq_stage[0], wq_bf, 0)
                w_cast(wq_stage[1], wq_bf, 1)
                x_chain_pe(2)
                w_cast(wq_stage[2], wq_bf, 2)
                w_cast(wq_stage[3], wq_bf, 3)
                x_chain_pe(3)
                for db in range(4, DBLK):
                    w_cast(wq_stage[db], wq_bf, db)
                # first q group db-OUTER over eb 0-5 (6 PSUM banks): starts
                # on the first Wq chunk and consumes them as they arrive,
                # instead of stalling until all of Wq has landed
                banks = [
                    psum.tile([P, 512], F32, tag="big", name=f"q0_{eb}")
                    for eb in range(6)
                ]
                for db in range(DBLK):
                    for eb in range(6):
                        nc.tensor.matmul(
                            banks[eb], lhsT=wq_bf[:, db, eb * P:(eb + 1) * P],
                            rhs=xT[:, db, 0:512],
                            start=(db == 0), stop=(db == DBLK - 1),
                        )
                for eb in range(6):
                    nc.any.tensor_copy(qT[:, eb, 0:512], banks[eb])
                # x tiles 4-7 arrived during the first q group; transpose now
                for tb in range(4, 8):
                    x_chain_pe(tb)
                # eb 6-7 finish db-inner (Wq fully resident by now)
                bank0 = psum.tile([P, 512], F32, tag="big", name="q0a")
                bank1 = psum.tile([P, 512], F32, tag="big", name="q0b")
                for db in range(DBLK):
                    nc.tensor.matmul(
                        bank0, lhsT=wq_bf[:, db, 6 * P:7 * P],
                        rhs=xT[:, db, 0:512],
                        start=(db == 0), stop=(db == DBLK - 1),
                    )
                    nc.tensor.matmul(
                        bank1, lhsT=wq_bf[:, db, 7 * P:8 * P],
                        rhs=xT[:, db, 0:512],
                        start=(db == 0), stop=(db == DBLK - 1),
                    )
                nc.any.tensor_copy(qT[:, 6, 0:512], bank0)
                nc.any.tensor_copy(qT[:, 7, 0:512], bank1)
                # Wk behind Wq on the sync ring; casts emitted here, well
                # after dispatch, data-paced. x tiles 8-15 dispatch behind
                # Wk, Wv last (its casts wait for Wq's SBUF slot anyway).
                for db in range(DBLK):
                    w_chain(Wk, wk_bf, db)
                late_stage = {tb: x_late_dma(tb) for tb in range(8, TB)}
                wv_stage = [w_dma(Wv, db) for db in range(DBLK)]
                qk_group(wq_bf, qT, 1)
                qk_group(wk_bf, kT, 0)
                # tb8-11 casts here (data landed during k-ts0) so their
                # xs32b slots recycle and x12-15 dispatch ahead of Wv
                bfs8 = [x_late_cast(late_stage[tb]) for tb in range(8, 12)]
                qk_group(wk_bf, kT, 1)
                # x tiles 8-15: quad-transposes (one 512-wide evacuation
                # per db instead of 4 small ones)
                x_quad_transpose(bfs8, 8)
                bfs12 = [x_late_cast(late_stage[tb]) for tb in range(12, TB)]
                x_quad_transpose(bfs12, 12)
                qk_group(wq_bf, qT, 2)
                qk_group(wk_bf, kT, 2)
                qk_group(wq_bf, qT, 3)
                # Wv casts here: Wq's wbf slot frees once q-ts3's matmuls
                # have read it (just above, earlier in the PE stream), and
                # the casts overlap k-ts3 on DVE/ACT
                for db in range(DBLK):
                    w_cast(wv_stage[db], wv_bf, db)
                qk_group(wk_bf, kT, 3)
                for tb in range(TB):
                    v_group(tb)
                # early scores for i-spans 0-1 (cheap in fp8): phase C then
                # starts straight on span-1 AV, and the span-0 E tiles that
                # gate the tail chain are long since resident
                eidx = 0
                for s in range(2):
                    for jb in range(4 * s + 4):
                        i0 = max(s * 512, jb * P)
                        L = (s + 1) * 512 - i0
                        score_exp(jb, i0, L, etE[:, eidx, 0:L])
                        eidx += 1

            # ================= Phase C+D: attention =====================
            # Span order 1,2,3,0: ending on span 0 (whose E tiles were
            # computed in phase B) makes the tail-gating AV chain the
            # 1-block ib=0 instead of a 13-block one.
            with tc.tile_pool(name="etp", bufs=16) as etp, \
                 tc.tile_pool(name="outp", bufs=8) as outp, \
                 tc.tile_pool(name="rsp", bufs=8) as rsp:
                ET_BASE = {0: 0, 1: 4}
                for s in (1, 2, 3, 0):
                    # --- scores + exp for i-span s, all jb <= 4s+3 ---
                    # (spans 0-1 were already computed inside phase B; see
                    # the early-scores fill)
                    et_tiles = []
                    et_i0 = []
                    for jb in range(4 * s + 4):
                        i0 = max(s * 512, jb * P)
                        L = (s + 1) * 512 - i0
                        if s < 2:
                            et = etE[:, ET_BASE[s] + jb, :]
                        else:
                            et = etp.tile([P, 512], BF16, tag="et")
                            score_exp(jb, i0, L, et[:, 0:L])
                        et_tiles.append(et)
                        et_i0.append(i0)

                    # --- AV + rowsums for the 4 i-blocks in span s ---
                    # span 0 (processed last) runs in reverse so the final
                    # AV chain (and hence the tail-gating output DMA) is the
                    # 1-block ib=0
                    ib_order = (
                        range(4 * s, 4 * s + 4) if s != 0
                        else range(3, -1, -1)
                    )
                    # tail block's rowsum+reciprocal hoisted ahead of the
                    # whole span: it depends only on et_tiles[0] (computed in
                    # phase B), so the final exposed chain is just
                    # 2 AV matmuls -> scales -> DMAs
                    tail_rsum = None
                    if s == 0:
                        pss = psum.tile([P, 1], F32, tag="small", bufs=2)
                        nc.tensor.matmul(
                            pss, lhsT=et_tiles[0][:, 0:P], rhs=ones,
                            start=True, stop=True,
                        )
                        tail_rsum = rsp.tile([P, 1], F32)
                        nc.vector.reciprocal(tail_rsum, pss)
                    for ib in ib_order:
                        tail = s == 0 and ib == 0
                        ps0 = psum.tile([P, 512], F32, tag="big")
                        ps1 = psum.tile([P, 512], F32, tag="big")
                        if not tail:
                            pss = psum.tile([P, 1], F32, tag="small", bufs=2)
                        for jb in range(ib + 1):
                            off = ib * P - et_i0[jb]
                            lhsT = et_tiles[jb][:, off:off + P]
                            first = jb == 0
                            last = jb == ib
                            nc.tensor.matmul(
                                ps0, lhsT=lhsT, rhs=vsb[:, jb, 0:512],
                                start=first, stop=last,
                            )
                            nc.tensor.matmul(
                                ps1, lhsT=lhsT, rhs=vsb[:, jb, 512:1024],
                                start=first, stop=last,
                            )
                            if not tail:
                                nc.tensor.matmul(
                                    pss, lhsT=lhsT, rhs=ones,
                                    start=first, stop=last,
                                )
                        if tail:
                            rsum = tail_rsum
                        else:
                            rsum = rsp.tile([P, 1], F32)
                            nc.vector.reciprocal(rsum, pss)
                        # the tail-gating block (s=0, ib=0, processed last)
                        # splits its scales+DMAs into 256-col chunks, pairs
                        # DVE+ACT per half, and spreads the DMA dispatches
                        # across engine rings so the final transfer isn't
                        # queued behind 3 other ~600ns dispatches on the
                        # sync sequencer
                        chunks = 2 if tail else 1
                        cw = 512 // chunks
                        tail_rings = [nc.gpsimd, nc.scalar, nc.sync, nc.gpsimd]
                        for es, ps in ((0, ps0), (1, ps1)):
                            ob = outp.tile([P, 512], F32)
                            for c in range(chunks):
                                sl = slice(c * cw, (c + 1) * cw)
                                # alternate DVE / ACT so the PSUM-freeing
                                # scale is never queued behind the other's
                                on_dve = (es == 0) if not tail else (c == 0)
                                if on_dve:
                                    nc.vector.tensor_scalar_mul(
                                        ob[:, sl], ps[:, sl], rsum
                                    )
                                else:
                                    nc.scalar.activation(
                                        ob[:, sl], ps[:, sl],
                                        mybir.ActivationFunctionType.Copy,
                                        scale=rsum,
                                    )
                                if tail:
                                    ring = tail_rings[2 * es + c]
                                else:
                                    # alternate sync/gpsimd so the tail's
                                    # dispatches never queue behind a
                                    # backlog on one ring
                                    ring = nc.sync if (ib + es) % 2 else nc.gpsimd
                                ring.dma_start(
                                    out=out[ib * P:(ib + 1) * P,
                                            es * 512 + c * cw:
                                            es * 512 + (c + 1) * cw],
                                    in_=ob[:, sl],
                                )
    return nc


_NC_CACHE = None


def _get_nc():
    global _NC_CACHE
    if _NC_CACHE is None:
        nc = bass.Bass(
            "TRN2", target_bir_lowering=False, debug=False, num_devices=1
        )
        _emit(nc)
        _split_multi_waits(nc)
        _NC_CACHE = nc
    return _NC_CACHE


def kernel(x, Wq, Wk, Wv):
    assert x.shape == (B, T, D), x.shape
    nc = _get_nc()
    Wq = np.ascontiguousarray(Wq, dtype=np.float32)
    Wk = np.ascontiguousarray(Wk, dtype=np.float32)
    Wv = np.ascontiguousarray(Wv, dtype=np.float32)
    in_maps = [
        {
            "x": np.ascontiguousarray(x[b], dtype=np.float32),
            "Wq": Wq,
            "Wk": Wk,
            "Wv": Wv,
        }
        for b in range(B)
    ]
    res = run_bass_kernel_spmd(nc, in_maps, core_ids=list(range(B)))
    out = np.stack([res.results[b]["out"] for b in range(B)], axis=0)
    kernel.last_exec_time_ns = res.exec_time_ns
    return out
